# revision 6
# baseline (speedup 1.0000x reference)
"""Bass/Trainium2 kernel for nn_Decoder (free-running LSTM decoder).

Math refactor (exact, done on host in fp32):
  reference step n (teacher forcing never fires, target unused):
    gates_n = y_n @ W_y.T + h_n @ W_hh.T + C0      C0 = c@W_c.T + b_ih + b_hh
    cell'   = sig(f)*cell + sig(i)*tanh(g)
    h'      = sig(o)*tanh(cell')
    y'      = h' @ Wh2o_h.T + y_const              y_const = c@Wh2o_c.T + h2o_b
  For n >= 1, y_n is an affine function of h_n, so
    gates_n = h_n @ W_eff.T + C1
    W_eff = W_hh + W_y @ Wh2o_h,  C1 = C0 + y_const @ W_y.T
  Host runs steps 0..N_HOST-1 in numpy; the device runs the remaining
  T_DEV steps of the pure h-recurrence, data-parallel over batch on 8 cores.

Device layout per core (batch shard of 64):
  gates PSUM G [64, 2048], gate order [g | i | f | o] (one 512-col bank each).
  Per step: 4 const-matmuls (identity trick, start=True) + 16 gate matmuls
  (f32r, lhsT = hT chunks [128,64], rhs = W_eff.T stream [128,512]) + 4
  y-matmuls; ACT does the 4 gate activations + tanh(cell); DVE does the cell
  update and h; 4 PE transposes rebuild hT for the next step.
"""

import sys

sys.path.insert(0, "/opt/trn_rl_repo")

import numpy as np

B, T, F, H = 512, 1024, 64, 512
NCORES = 8
BL = B // NCORES            # 64 batch rows per core
U = 14                      # steps per For_i iteration
T_DEV = 1022                # device steps; 1022 = 73 * 14
NITER = T_DEV // U
N_HOST = T - T_DEV

G4 = 4 * H                  # 2048
# gate order in the device layout: [g, i, f, o]; original rows are [i, f, g, o]
_PERM = np.concatenate([
    np.arange(2 * H, 3 * H),      # g
    np.arange(0, H),              # i
    np.arange(H, 2 * H),          # f
    np.arange(3 * H, 4 * H),      # o
])

_BASS_CACHE = {}


def _sigmoid(x):
    return 1.0 / (1.0 + np.exp(-x))


def _build_bass():
    key = (T_DEV, U)
    if key in _BASS_CACHE:
        return _BASS_CACHE[key]
    from concourse import bacc, tile, mybir

    F32R = mybir.dt.float32r
    F32 = mybir.dt.float32
    ACTF = mybir.ActivationFunctionType

    nc = bacc.Bacc()
    d_ht0 = nc.declare_dram_parameter("ht0", [128, 256], F32R, isOutput=False)
    d_cell0 = nc.declare_dram_parameter("cell0", [64, 512], F32, isOutput=False)
    d_ws = nc.declare_dram_parameter("ws", [128, 4 * G4], F32R, isOutput=False)
    d_cst = nc.declare_dram_parameter("cst", [64, G4], F32R, isOutput=False)
    d_wh2o = nc.declare_dram_parameter("wh2o", [128, 256], F32R, isOutput=False)
    d_ycst = nc.declare_dram_parameter("ycst", [64, 64], F32, isOutput=False)
    d_id = nc.declare_dram_parameter("ident", [64, 64], F32R, isOutput=False)
    d_out = nc.declare_dram_parameter("out", [64, T_DEV, 64], F32, isOutput=True)

    import concourse.bass as bass

    with tile.TileContext(nc) as tc:
        with (
            tc.tile_pool(name="wpool", bufs=1) as wpool,
            tc.tile_pool(name="state", bufs=1) as state,
            tc.tile_pool(name="work", bufs=2) as work,
            tc.tile_pool(name="ypool", bufs=2) as ypool,
            tc.tile_pool(name="gps", bufs=1, space="PSUM") as gps,
            tc.tile_pool(name="yps", bufs=2, space="PSUM") as yps,
            tc.tile_pool(name="tps", bufs=2, space="PSUM") as tps,
        ):
            ws_t = wpool.tile([128, 4 * G4], F32R)
            cst_t = wpool.tile([64, G4], F32R)
            wh2o_t = wpool.tile([128, 256], F32R)
            ycst_t = wpool.tile([64, 64], F32)
            id_t = wpool.tile([64, 64], F32R)
            nc.gpsimd.dma_start(ws_t[:], d_ws[:])
            nc.gpsimd.dma_start(cst_t[:], d_cst[:])
            nc.gpsimd.dma_start(wh2o_t[:], d_wh2o[:])
            nc.gpsimd.dma_start(ycst_t[:], d_ycst[:])
            nc.gpsimd.dma_start(id_t[:], d_id[:])

            # ping-pong state
            ht = [state.tile([128, 256], F32R, name=f"ht{p}") for p in (0, 1)]
            cell = [state.tile([64, 512], F32, name=f"cell{p}") for p in (0, 1)]
            nc.gpsimd.dma_start(ht[0][:], d_ht0[:])
            nc.gpsimd.dma_start(cell[0][:], d_cell0[:])

            G = gps.tile([64, G4], F32)

            def step(u):
                cur, nxt = u % 2, 1 - (u % 2)
                ht_c = ht[cur]
                # const init of all four banks (identity trick)
                for j in range(4):
                    nc.tensor.matmul(
                        G[:, j * 512:(j + 1) * 512], id_t[:],
                        cst_t[:, j * 512:(j + 1) * 512],
                        start=True, stop=False,
                    )
                # gates: bank j, contraction over 4 k-tiles
                for j in range(4):
                    for k in range(4):
                        nc.tensor.matmul(
                            G[:, j * 512:(j + 1) * 512],
                            ht_c[:, k * 64:(k + 1) * 64],
                            ws_t[:, k * G4 + j * 512: k * G4 + (j + 1) * 512],
                            start=False, stop=(k == 3),
                        )
                # activations: banks [g, i, f, o]
                sa = work.tile([64, G4], F32, name="sa")
                nc.scalar.activation(sa[:, 0:512], G[:, 0:512], ACTF.Tanh)
                nc.scalar.activation(sa[:, 512:1024], G[:, 512:1024], ACTF.Sigmoid)
                nc.scalar.activation(sa[:, 1024:1536], G[:, 1024:1536], ACTF.Sigmoid)
                nc.scalar.activation(sa[:, 1536:2048], G[:, 1536:2048], ACTF.Sigmoid)
                # cell update
                t1 = work.tile([64, 512], F32, name="t1")
                nc.vector.tensor_mul(t1[:], sa[:, 512:1024], sa[:, 0:512])
                t2 = work.tile([64, 512], F32, name="t2")
                nc.vector.tensor_mul(t2[:], sa[:, 1024:1536], cell[cur][:])
                nc.vector.tensor_add(cell[nxt][:], t1[:], t2[:])
                tc_t = work.tile([64, 512], F32, name="tc")
                nc.scalar.activation(tc_t[:], cell[nxt][:], ACTF.Tanh)
                h_t = work.tile([64, 512], F32R, name="h")
                nc.vector.tensor_mul(h_t[:], sa[:, 1536:2048], tc_t[:])
                # rebuild hT for next step: 4 PE transposes + copies
                for k in range(4):
                    tp = tps.tile([128, 64], F32R, name="tp")
                    nc.tensor.transpose(tp[:], h_t[:, k * 128:(k + 1) * 128], id_t[:])
                    if k % 2 == 0:
                        nc.scalar.copy(ht[nxt][:, k * 64:(k + 1) * 64], tp[:])
                    else:
                        nc.vector.tensor_copy(ht[nxt][:, k * 64:(k + 1) * 64], tp[:])
                # y = h_new @ Wh2o_h.T (reads the freshly built hT)
                yps_t = yps.tile([64, 64], F32)
                for k in range(4):
                    nc.tensor.matmul(
                        yps_t[:], ht[nxt][:, k * 64:(k + 1) * 64],
                        wh2o_t[:, k * 64:(k + 1) * 64],
                        start=(k == 0), stop=(k == 3),
                    )
                return yps_t

            tc.strict_bb_all_engine_barrier()
            with tc.For_i(
                0, NITER, 1,
                hint_engines=(mybir.EngineType.PE, mybir.EngineType.Activation,
                              mybir.EngineType.DVE),
            ) as it:
                sy = ypool.tile([64, U, 64], F32, name="sy")
                for u in range(U):
                    yps_t = step(u)
                    nc.vector.tensor_add(sy[:, u, :], yps_t[:], ycst_t[:])
                nc.gpsimd.dma_start(d_out[:, bass.ds(it * U, U), :], sy[:])

    nc.compile()
    _BASS_CACHE[key] = nc
    return nc


def _host_prep(c, V_w, V_b, W_ih, W_hh, b_ih, b_hh, h2o_w, h2o_b):
    """Run N_HOST steps in numpy; return ys prefix and device operands."""
    W_y = W_ih[:, :F]                   # [4H, F]
    W_c = W_ih[:, F:]                   # [4H, H]
    Wh2o_h = h2o_w[:, :H]               # [F, H]
    Wh2o_c = h2o_w[:, H:]               # [F, H]
    y_const = c @ Wh2o_c.T + h2o_b      # [B, F]
    C0 = c @ W_c.T + b_ih + b_hh        # [B, 4H]
    W_eff = W_hh + W_y @ Wh2o_h         # [4H, H]
    C1 = C0 + y_const @ W_y.T           # [B, 4H]

    h = np.tanh(c @ V_w.T + V_b)
    cell = h.copy()
    y = np.zeros((B, F), np.float32)
    ys_prefix = np.zeros((B, N_HOST, F), np.float32)
    for n in range(N_HOST):
        gates = y @ W_y.T + h @ W_hh.T + C0
        i_g, f_g, g_g, o_g = np.split(gates, 4, axis=1)
        cell = _sigmoid(f_g) * cell + _sigmoid(i_g) * np.tanh(g_g)
        h = _sigmoid(o_g) * np.tanh(cell)
        y = h @ Wh2o_h.T + y_const
        ys_prefix[:, n] = y

    # device operand prep (shared across cores)
    W_eff_p = W_eff[_PERM]              # [2048, 512] gate order [g,i,f,o]
    C1_p = C1[:, _PERM]                 # [B, 2048]
    # WS[p, k*2048 + n] = W_eff_p[n, k*128 + p]
    WS = np.ascontiguousarray(
        W_eff_p.T.reshape(4, 128, G4).transpose(1, 0, 2).reshape(128, 4 * G4)
    )
    # WH2O[p, k*64 + f] = Wh2o_h[f, k*128 + p]
    WH2O = np.ascontiguousarray(
        Wh2o_h.T.reshape(4, 128, F).transpose(1, 0, 2).reshape(128, 4 * F)
    )
    I64 = np.eye(64, dtype=np.float32)
    return ys_prefix, h, cell, y_const, C1_p, WS, WH2O, I64


def kernel(**inputs):
    from concourse.bass_utils import run_bass_kernel_spmd

    c = np.asarray(inputs["c"], np.float32)
    V_w, V_b = np.asarray(inputs["V_w"], np.float32), np.asarray(inputs["V_b"], np.float32)
    W_ih, W_hh = np.asarray(inputs["W_ih"], np.float32), np.asarray(inputs["W_hh"], np.float32)
    b_ih, b_hh = np.asarray(inputs["b_ih"], np.float32), np.asarray(inputs["b_hh"], np.float32)
    h2o_w, h2o_b = np.asarray(inputs["h2o_w"], np.float32), np.asarray(inputs["h2o_b"], np.float32)

    ys_prefix, h, cell, y_const, C1_p, WS, WH2O, I64 = _host_prep(
        c, V_w, V_b, W_ih, W_hh, b_ih, b_hh, h2o_w, h2o_b
    )

    nc = _build_bass()
    in_maps = []
    for core in range(NCORES):
        sl = slice(core * BL, (core + 1) * BL)
        hT = h[sl].T.copy()             # [512, 64]
        ht0 = np.ascontiguousarray(
            hT.reshape(4, 128, BL).transpose(1, 0, 2).reshape(128, 4 * BL)
        )
        in_maps.append({
            "ht0": ht0,
            "cell0": np.ascontiguousarray(cell[sl]),
            "ws": WS,
            "cst": np.ascontiguousarray(C1_p[sl]),
            "wh2o": WH2O,
            "ycst": np.ascontiguousarray(y_const[sl]),
            "ident": I64,
        })

    res = run_bass_kernel_spmd(nc, in_maps, list(range(NCORES)))

    out = np.zeros((B, T, F), np.float32)
    out[:, :N_HOST] = ys_prefix
    for core in range(NCORES):
        out[core * BL:(core + 1) * BL, N_HOST:N_HOST + T_DEV] = res.results[core]["out"]
    return out


# revision 19
# speedup vs baseline: 89.1128x; 89.1128x over previous
"""Bass/Trainium2 kernel for nn_Decoder (free-running LSTM decoder).

Math refactor (exact, done on host in fp32):
  reference step n (teacher forcing never fires, target unused):
    gates_n = y_n @ W_y.T + h_n @ W_hh.T + C0      C0 = c@W_c.T + b_ih + b_hh
    cell'   = sig(f)*cell + sig(i)*tanh(g)
    h'      = sig(o)*tanh(cell')
    y'      = h' @ Wh2o_h.T + y_const              y_const = c@Wh2o_c.T + h2o_b
  For n >= 1, y_n is an affine function of h_n, so
    gates_n = h_n @ W_eff.T + C1
    W_eff = W_hh + W_y @ Wh2o_h,  C1 = C0 + y_const @ W_y.T
  Host runs steps 0..N_HOST-1 in numpy; the device runs the remaining
  T_DEV steps of the pure h-recurrence, data-parallel over batch on 8 cores.

Device layout per core (batch shard of 64):
  gates PSUM G [64, 2048], gate order [g | i | f | o] (one 512-col bank each).
  Per step: 4 const-matmuls (identity trick, start=True) + 16 gate matmuls
  (f32r, lhsT = hT chunks [128,64], rhs = W_eff.T stream [128,512]) + 4
  y-matmuls; ACT does the 4 gate activations + tanh(cell); DVE does the cell
  update and h; 4 PE transposes rebuild hT for the next step.
"""

import sys

sys.path.insert(0, "/opt/trn_rl_repo")

import numpy as np

B, T, F, H = 512, 1024, 64, 512
NCORES = 8
BL = B // NCORES            # 64 batch rows per core
U = 14                      # steps per For_i iteration
T_DEV = 1022                # device steps; 1022 = 73 * 14
NITER = T_DEV // U
N_HOST = T - T_DEV
STATIC_UNROLL = False
STAGGERED = True

G4 = 4 * H                  # 2048
# gate order in the device layout: [g, f, i, o]; original rows are [i, f, g, o]
_PERM = np.concatenate([
    np.arange(2 * H, 3 * H),      # g
    np.arange(H, 2 * H),          # f
    np.arange(0, H),              # i
    np.arange(3 * H, 4 * H),      # o
])

_BASS_CACHE = {}


def _sigmoid(x):
    return 1.0 / (1.0 + np.exp(-x))


def _build_bass():
    key = (T_DEV, U, STATIC_UNROLL, STAGGERED)
    if key in _BASS_CACHE:
        return _BASS_CACHE[key]
    from concourse import bacc, tile, mybir

    F32R = mybir.dt.float32r
    F32 = mybir.dt.float32
    ACTF = mybir.ActivationFunctionType

    nc = bacc.Bacc()
    d_ht0 = nc.declare_dram_parameter("ht0", [128, 256], F32R, isOutput=False)
    d_cell0 = nc.declare_dram_parameter("cell0", [64, 512], F32, isOutput=False)
    d_ws = nc.declare_dram_parameter("ws", [128, 4 * G4], F32R, isOutput=False)
    d_cst = nc.declare_dram_parameter("cst", [64, G4], F32R, isOutput=False)
    d_wh2o = nc.declare_dram_parameter("wh2o", [128, 256], F32R, isOutput=False)
    d_ycst = nc.declare_dram_parameter("ycst", [64, 64], F32, isOutput=False)
    d_id = nc.declare_dram_parameter("ident", [64, 64], F32R, isOutput=False)
    d_h0p = nc.declare_dram_parameter("h0p", [64, 512], F32R, isOutput=False)
    d_out = nc.declare_dram_parameter("out", [64, T_DEV, 64], F32, isOutput=True)

    import concourse.bass as bass

    with tile.TileContext(nc) as tc:
        with (
            tc.tile_pool(name="wpool", bufs=1) as wpool,
            tc.tile_pool(name="state", bufs=1) as state,
            tc.tile_pool(name="work", bufs=2) as work,
            tc.tile_pool(name="ypool", bufs=2) as ypool,
            tc.tile_pool(name="gps", bufs=1, space="PSUM") as gps,
            tc.tile_pool(name="yps", bufs=1, space="PSUM") as yps,
            tc.tile_pool(name="tps", bufs=3, space="PSUM") as tps,
        ):
            ws_t = wpool.tile([128, 4 * G4], F32R)
            cst_t = wpool.tile([64, G4], F32R)
            wh2o_t = wpool.tile([128, 256], F32R)
            ycst_t = wpool.tile([64, 64], F32)
            id_t = wpool.tile([64, 64], F32R)
            nc.gpsimd.dma_start(ws_t[:], d_ws[:])
            nc.gpsimd.dma_start(cst_t[:], d_cst[:])
            nc.gpsimd.dma_start(wh2o_t[:], d_wh2o[:])
            nc.gpsimd.dma_start(ycst_t[:], d_ycst[:])
            nc.gpsimd.dma_start(id_t[:], d_id[:])

            # state: cell ping-pong, h ping-pong (to form deltas), dhT chunks
            ht0_t = state.tile([128, 256], F32R, name="ht_init")
            cell = [state.tile([64, 512], F32, name=f"cell{p}") for p in (0, 1)]
            hp = [state.tile([64, 512], F32R, name=f"h{p}") for p in (0, 1)]
            dht = state.tile([128, 256], F32R, name="dht")
            nc.gpsimd.dma_start(ht0_t[:], d_ht0[:])
            nc.gpsimd.dma_start(cell[0][:], d_cell0[:])
            nc.gpsimd.dma_start(hp[0][:], d_h0p[:])

            # persistent PSUM accumulators: gates banks + y
            Gb = [gps.tile([64, 512], F32, name=f"g{j}") for j in range(4)]
            Yp = yps.tile([64, 64], F32)

            # ---- one-time PSUM init: gates = h0@W_eff.T + C1; y = h0@Wh2o.T
            for j in range(4):
                nc.tensor.matmul(
                    Gb[j][:], id_t[:], cst_t[:, j * 512:(j + 1) * 512],
                    start=True, stop=False, skip_group_check=True,
                )
                for k in range(4):
                    nc.tensor.matmul(
                        Gb[j][:], ht0_t[:, k * 64:(k + 1) * 64],
                        ws_t[:, k * G4 + j * 512: k * G4 + (j + 1) * 512],
                        start=False, stop=False, skip_group_check=True,
                    )
            for k in range(4):
                nc.tensor.matmul(
                    Yp[:], ht0_t[:, k * 64:(k + 1) * 64],
                    wh2o_t[:, k * 64:(k + 1) * 64],
                    start=(k == 0), stop=False, skip_group_check=True,
                )

            def step(u, sy):
                cur, nxt = u % 2, 1 - (u % 2)
                # activations on the current gate banks [g, f, i, o]
                sa = work.tile([64, G4], F32, name="sa")
                nc.scalar.activation(sa[:, 0:512], Gb[0][:], ACTF.Tanh)
                nc.scalar.activation(sa[:, 512:1024], Gb[1][:], ACTF.Sigmoid)
                nc.scalar.activation(sa[:, 1024:1536], Gb[2][:], ACTF.Sigmoid)
                nc.scalar.activation(sa[:, 1536:1792], Gb[3][:, 0:256], ACTF.Sigmoid)
                nc.scalar.activation(sa[:, 1792:2048], Gb[3][:, 256:512], ACTF.Sigmoid)
                # cell/h update in halves; build dh = h_new - h_prev; transpose it
                for hh in (0, 1):
                    s = slice(hh * 256, (hh + 1) * 256)
                    t2 = work.tile([64, 256], F32, name=f"t2{hh}")
                    nc.vector.tensor_mul(t2[:], sa[:, 512 + hh * 256:768 + hh * 256],
                                         cell[cur][:, s])
                    t1 = work.tile([64, 256], F32, name=f"t1{hh}")
                    nc.vector.tensor_mul(t1[:], sa[:, 1024 + hh * 256:1280 + hh * 256],
                                         sa[:, 0 + hh * 256:256 + hh * 256])
                    nc.vector.tensor_add(cell[nxt][:, s], t1[:], t2[:])
                    tc_t = work.tile([64, 256], F32, name=f"tc{hh}")
                    nc.scalar.activation(tc_t[:], cell[nxt][:, s], ACTF.Tanh)
                    nc.vector.tensor_mul(hp[nxt][:, s],
                                         sa[:, 1536 + hh * 256:1792 + hh * 256], tc_t[:])
                    dh = work.tile([64, 256], F32R, name=f"dh{hh}")
                    nc.vector.tensor_sub(dh[:], hp[nxt][:, s], hp[cur][:, s])
                    for kk in (0, 1):
                        k = 2 * hh + kk
                        tp = tps.tile([128, 64], F32R, name="tp")
                        nc.tensor.transpose(tp[:], dh[:, kk * 128:(kk + 1) * 128], id_t[:])
                        if k % 2 == 0:
                            nc.scalar.copy(dht[:, k * 64:(k + 1) * 64], tp[:])
                        else:
                            nc.vector.tensor_copy(dht[:, k * 64:(k + 1) * 64], tp[:])
                # accumulate gate banks += dh @ W_eff.T in two k-passes:
                # pass A (k=0,1) only needs the first half of dh
                def gmm(j, k):
                    nc.tensor.matmul(
                        Gb[j][:], dht[:, k * 64:(k + 1) * 64],
                        ws_t[:, k * G4 + j * 512: k * G4 + (j + 1) * 512],
                        start=False, stop=False, skip_group_check=True,
                    )

                def ymm(k):
                    nc.tensor.matmul(
                        Yp[:], dht[:, k * 64:(k + 1) * 64],
                        wh2o_t[:, k * 64:(k + 1) * 64],
                        start=False, stop=False, skip_group_check=True,
                    )

                for j in range(4):
                    for k in range(4):
                        gmm(j, k)
                for k in range(4):
                    ymm(k)
                nc.vector.tensor_add(sy[:, u, :], Yp[:], ycst_t[:])

            def iteration(itv):
                sy = ypool.tile([64, U, 64], F32, name="sy")
                for u in range(U):
                    step(u, sy)
                nc.gpsimd.dma_start(d_out[:, bass.ds(itv * U, U), :], sy[:])

            tc.strict_bb_all_engine_barrier()
            if STATIC_UNROLL:
                for itv in range(NITER):
                    iteration(itv)
            else:
                with tc.For_i(
                    0, NITER, 1,
                    hint_engines=(mybir.EngineType.PE, mybir.EngineType.Activation,
                                  mybir.EngineType.DVE),
                    staggered_reset=STAGGERED,
                ) as it:
                    iteration(it)

    nc.compile()
    _BASS_CACHE[key] = nc
    return nc


def _host_prep(c, V_w, V_b, W_ih, W_hh, b_ih, b_hh, h2o_w, h2o_b):
    """Run N_HOST steps in numpy; return ys prefix and device operands."""
    W_y = W_ih[:, :F]                   # [4H, F]
    W_c = W_ih[:, F:]                   # [4H, H]
    Wh2o_h = h2o_w[:, :H]               # [F, H]
    Wh2o_c = h2o_w[:, H:]               # [F, H]
    y_const = c @ Wh2o_c.T + h2o_b      # [B, F]
    C0 = c @ W_c.T + b_ih + b_hh        # [B, 4H]
    W_eff = W_hh + W_y @ Wh2o_h         # [4H, H]
    C1 = C0 + y_const @ W_y.T           # [B, 4H]

    h = np.tanh(c @ V_w.T + V_b)
    cell = h.copy()
    y = np.zeros((B, F), np.float32)
    ys_prefix = np.zeros((B, N_HOST, F), np.float32)
    for n in range(N_HOST):
        gates = y @ W_y.T + h @ W_hh.T + C0
        i_g, f_g, g_g, o_g = np.split(gates, 4, axis=1)
        cell = _sigmoid(f_g) * cell + _sigmoid(i_g) * np.tanh(g_g)
        h = _sigmoid(o_g) * np.tanh(cell)
        y = h @ Wh2o_h.T + y_const
        ys_prefix[:, n] = y

    # device operand prep (shared across cores)
    W_eff_p = W_eff[_PERM]              # [2048, 512] gate order [g,i,f,o]
    C1_p = C1[:, _PERM]                 # [B, 2048]
    # WS[p, k*2048 + n] = W_eff_p[n, k*128 + p]
    WS = np.ascontiguousarray(
        W_eff_p.T.reshape(4, 128, G4).transpose(1, 0, 2).reshape(128, 4 * G4)
    )
    # WH2O[p, k*64 + f] = Wh2o_h[f, k*128 + p]
    WH2O = np.ascontiguousarray(
        Wh2o_h.T.reshape(4, 128, F).transpose(1, 0, 2).reshape(128, 4 * F)
    )
    I64 = np.eye(64, dtype=np.float32)
    return ys_prefix, h, cell, y_const, C1_p, WS, WH2O, I64


def kernel(**inputs):
    from concourse.bass_utils import run_bass_kernel_spmd

    c = np.asarray(inputs["c"], np.float32)
    V_w, V_b = np.asarray(inputs["V_w"], np.float32), np.asarray(inputs["V_b"], np.float32)
    W_ih, W_hh = np.asarray(inputs["W_ih"], np.float32), np.asarray(inputs["W_hh"], np.float32)
    b_ih, b_hh = np.asarray(inputs["b_ih"], np.float32), np.asarray(inputs["b_hh"], np.float32)
    h2o_w, h2o_b = np.asarray(inputs["h2o_w"], np.float32), np.asarray(inputs["h2o_b"], np.float32)

    ys_prefix, h, cell, y_const, C1_p, WS, WH2O, I64 = _host_prep(
        c, V_w, V_b, W_ih, W_hh, b_ih, b_hh, h2o_w, h2o_b
    )

    nc = _build_bass()
    in_maps = []
    for core in range(NCORES):
        sl = slice(core * BL, (core + 1) * BL)
        hT = h[sl].T.copy()             # [512, 64]
        ht0 = np.ascontiguousarray(
            hT.reshape(4, 128, BL).transpose(1, 0, 2).reshape(128, 4 * BL)
        )
        in_maps.append({
            "ht0": ht0,
            "cell0": np.ascontiguousarray(cell[sl]),
            "ws": WS,
            "cst": np.ascontiguousarray(C1_p[sl]),
            "wh2o": WH2O,
            "ycst": np.ascontiguousarray(y_const[sl]),
            "ident": I64,
            "h0p": np.ascontiguousarray(h[sl]),
        })

    res = run_bass_kernel_spmd(nc, in_maps, list(range(NCORES)))

    out = np.zeros((B, T, F), np.float32)
    out[:, :N_HOST] = ys_prefix
    for core in range(NCORES):
        out[core * BL:(core + 1) * BL, N_HOST:N_HOST + T_DEV] = res.results[core]["out"]
    return out
